# revision 33
# baseline (speedup 1.0000x reference)
"""Multi-head local (windowed) attention on 8 Trainium2 NeuronCores.

Reference computation (fp32):
  Q/K/V = x @ W{q,k,v}.T + b{q,k,v}            x: [B=4, L=8192, D=1024]
  per window of 128 tokens, per head (H=16, dk=64):
    S = Q K^T / sqrt(dk); P = softmax(S); att = P V
  out = att @ Wo.T + bo
Sharding: data-parallel over the flattened (B*L) token axis — each of the 8
cores gets 4096 tokens = 32 windows (window boundaries align with the split).
Weights replicated; host pre-transposes x / weights, post-transposes output.

Performance structure (751551ns fp32r baseline -> 436109ns, rel 4.7e-3):
  - All four projections run as fp8e4m3 DoubleRow matmuls (0.5 PE
    cycles/row, two 128-deep contraction tiles per instruction) with a
    3-term error-compensated split: x@W ~= xh@Wh + xh@Wl + xl@Wh where
    xh=fp8(x), xl=fp8(x-xh) and Wh/Wl likewise for 32*W^T (the 32x scale
    keeps the tiny 1/sqrt(D)-scaled weights out of fp8's subnormal range;
    it is divided back out in the bias activations). Measured projection
    error 0.13% — better than bf16's 0.24%. Q/K/V split on the host; the
    attention output's hi/lo split happens on DVE after each transpose
    (copy + scalar_tensor_tensor subtract), and Wo carries its own 32x so
    the y activation de-scales by 1/1024.
  - Attention (scores, exp, PV, transposes) runs in bf16 (1 cycle/row at
    any free size). PSUM accumulation, biases, softmax sums, y stay fp32.
  - Three-stage software pipeline: emission round st interleaves
    projections(st) + attention chains(st-1) + O-projection(st-2) in the
    PE stream (engines execute in program order, so emission order IS the
    schedule). Chain units are merged proportionally by PE weight between
    the dense projection units, with an s1->s2 lag so the S -> exp(ScalarE)
    -> PV round-trip is hidden. Steady-state PE occupancy ~100%.
  - Q/K/O biases ride ScalarE (activation Identity with per-partition bias
    AP + 1/32 de-scale; Exp and Identity share one act table) so DVE only
    does V packing, 1/l normalization and transpose copies.
  - DMAs are few and large, queued in consumption order: x0h | wqh(chunks)
    | bias | x0l | wql | x1 | wk | wv | wo. Round 0 runs Q(0) as two
    kk-major quads (terms ordered by operand arrival, two psum groups
    borrowed from idle ps_s banks) plus Q/K of supertile 1, so the PE
    consumes weight chunks as they stream in; every later round carries
    V(st) + QK(st+1). y leaves in quarters (eighths at the end) so the
    final drain only waits a small store.

Layouts: Q/K feature-major via matmul(W^T, x^T); V token-major via swapped
operands, augmented per head with ones columns [V_h (64) | 1 1] so the PV
matmul also emits softmax sums l[q] at psum columns 64:66; P' = exp(S^T)
serves directly as the PV lhsT; normalization is a per-partition
tensor_scalar multiply by 1/l; only the attention output is PE-transposed
back to feature-major; V bias is folded into the output bias on the host
(softmax rows sum to one). The 32x V scale cancels in the y activation.

Note: independent matmul accumulation groups must NOT share a PSUM bank on
real hardware — psum pools are bank-granular (ps_proj 2 + ps_s 3 + ps_pv 2
+ ps_tr 1 = 8 banks).
"""

import sys

sys.path.insert(0, "/opt/trn_rl_repo")

from contextlib import ExitStack

import numpy as np

import concourse.bass as bass  # noqa: F401
import concourse.tile as tile
from concourse import bacc, mybir
from concourse.bass_utils import run_bass_kernel_spmd

DT = mybir.dt
AFT = mybir.ActivationFunctionType
DR = mybir.MatmulPerfMode.DoubleRow

N_CORES = 8
D = 1024  # model dim
H = 16  # heads
DK = 64  # head dim
W = 128  # window size
TC = 4096  # tokens per core
T = 256  # tokens per supertile (2 windows)
NST = TC // T  # supertiles per core
NWIN = T // W  # windows per supertile
NCH = D // 128  # 128-row feature chunks
NK2 = NCH // 2  # DoubleRow contraction-pair count
HA = DK + 2  # augmented V columns per head: [V_h (64) | ones (2)]
BF = DT.bfloat16
F8 = DT.float8e4
WSCALE = 32.0  # fp8 weight pre-scale (de-scaled in the bias activations)

_NC_CACHE = {}


def _build(lag=3, tr_lag=0, ps_proj_bufs=2, ps_s_bufs=3, o_shift=1):
    """Build + compile the single-core SPMD Bass program."""
    nc = bacc.Bacc("TRN2", target_bir_lowering=False, debug=False, num_devices=N_CORES)

    xT8 = {
        s: nc.dram_tensor(f"x{s}T", [D, TC], F8, kind="ExternalInput").ap()
        for s in "hl"
    }
    wT8 = {
        (p, s): nc.dram_tensor(f"w{p}{s}T", [D, D], F8, kind="ExternalInput").ap()
        for p in "qkv"
        for s in "hl"
    }
    woT8 = {
        s: nc.dram_tensor(f"wo{s}T", [D, D], F8, kind="ExternalInput").ap()
        for s in "hl"
    }
    biasT = nc.dram_tensor("bias", [128, 3, NCH], DT.float32, kind="ExternalInput").ap()
    onesT = nc.dram_tensor("ones", [128, 2], BF, kind="ExternalInput").ap()
    identT = nc.dram_tensor("ident", [128, 128], BF, kind="ExternalInput").ap()
    yT = nc.dram_tensor("yT", [D, TC], DT.float32, kind="ExternalOutput").ap()

    with tile.TileContext(nc) as tc, ExitStack() as ctx:
        wpool = ctx.enter_context(tc.tile_pool(name="w", bufs=1))
        const = ctx.enter_context(tc.tile_pool(name="const", bufs=1))
        xpool = ctx.enter_context(tc.tile_pool(name="x", bufs=2))
        qkpool = ctx.enter_context(tc.tile_pool(name="qk", bufs=3))
        vtokp = ctx.enter_context(tc.tile_pool(name="vtok", bufs=2))
        atokp = ctx.enter_context(tc.tile_pool(name="atok", bufs=2))
        attp = ctx.enter_context(tc.tile_pool(name="attT", bufs=2))
        ppool = ctx.enter_context(tc.tile_pool(name="p", bufs=8))
        rpool = ctx.enter_context(tc.tile_pool(name="r", bufs=8))
        ypool = ctx.enter_context(tc.tile_pool(name="y", bufs=2))
        ps_proj = ctx.enter_context(
            tc.tile_pool(name="ps_proj", bufs=ps_proj_bufs, space="PSUM")
        )
        ps_s = ctx.enter_context(tc.tile_pool(name="ps_s", bufs=ps_s_bufs, space="PSUM"))
        ps_pv = ctx.enter_context(tc.tile_pool(name="ps_pv", bufs=2, space="PSUM"))
        ps_tr = ctx.enter_context(tc.tile_pool(name="ps_tr", bufs=1, space="PSUM"))

        # ---- resident weights + consts, DMA-ordered by first use
        wt = {}  # ("q"/"k"/"v", "h"/"l") -> fp8 tile; "o" -> bf16 tile

        def alloc_w8(p, s):
            wt[p, s] = wpool.tile([128, NCH * D], F8, tag=f"w{p}{s}", name=f"w{p}{s}")

        def load_w8_chunk(p, s, kk):
            nc.sync.dma_start(
                wt[p, s][:, kk * D : (kk + 1) * D],
                wT8[p, s][kk * 128 : (kk + 1) * 128, :],
            )

        def load_w8(p, s):
            alloc_w8(p, s)
            nc.sync.dma_start(
                wt[p, s][:].rearrange("p (k c) -> p k c", c=D),
                wT8[p, s].rearrange("(k p) c -> p k c", p=128),
            )

        xts = {}  # (st, "h"/"l") -> fp8 tile [128, NCH*T]

        def load_x(st):
            for s in "hl":
                t = xpool.tile([128, NCH * T], F8, tag=f"x{s}", name=f"x{s}_{st}")
                nc.sync.dma_start(
                    t[:].rearrange("p (k t) -> p k t", t=T),
                    xT8[s].rearrange("(k p) t -> p k t", p=128)[
                        :, :, st * T : (st + 1) * T
                    ],
                )
                xts[st, s] = t

        # round 0 consumes wq chunk-wise as it streams in
        alloc_w8("q", "h")
        alloc_w8("q", "l")
        alloc_w8("k", "h")
        alloc_w8("k", "l")
        xts[0, "h"] = xpool.tile([128, NCH * T], F8, tag="xh", name="xh_0")
        xts[0, "l"] = xpool.tile([128, NCH * T], F8, tag="xl", name="xl_0")
        nc.sync.dma_start(
            xts[0, "h"][:].rearrange("p (k t) -> p k t", t=T),
            xT8["h"].rearrange("(k p) t -> p k t", p=128)[:, :, :T],
        )
        for kk in range(3):
            load_w8_chunk("q", "h", kk)
        bias_sb = const.tile([128, 3, NCH], DT.float32, tag="bias")
        nc.sync.dma_start(bias_sb[:], biasT)
        nc.sync.dma_start(
            xts[0, "l"][:].rearrange("p (k t) -> p k t", t=T),
            xT8["l"].rearrange("(k p) t -> p k t", p=128)[:, :, :T],
        )
        for kk in range(3, NCH):
            load_w8_chunk("q", "h", kk)
        for kk in range(NCH):
            load_w8_chunk("q", "l", kk)
        load_x(1)
        nc.sync.dma_start(
            wt["k", "h"][:].rearrange("p (k c) -> p k c", c=D),
            wT8["k", "h"].rearrange("(k p) c -> p k c", p=128),
        )
        nc.sync.dma_start(
            wt["k", "l"][:].rearrange("p (k c) -> p k c", c=D),
            wT8["k", "l"].rearrange("(k p) c -> p k c", p=128),
        )
        ones_sb = const.tile([128, 2], BF, tag="ones")
        nc.sync.dma_start(ones_sb[:], onesT)
        id_sb = const.tile([128, 128], BF, tag="ident")
        nc.sync.dma_start(id_sb[:], identT)
        load_w8("v", "h")
        load_w8("v", "l")
        for s in "hl":
            wt["o", s] = wpool.tile(
                [128, NCH * D], F8, tag=f"wo{s}", name=f"wo{s}"
            )
            nc.sync.dma_start(
                wt["o", s][:].rearrange("p (k c) -> p k c", c=D),
                woT8[s].rearrange("(k p) c -> p k c", p=128),
            )

        def w3(p, s):
            return wt[p, s][:].rearrange("p (k c) -> p k c", c=D)

        def x3(st, s):
            return xts[st, s][:].rearrange("p (k t) -> p k t", t=T)

        # fp8 DoubleRow term list: (x side, w side); TERMS0 matches the
        # round-0 DMA arrival order (x_lo lands before the w_lo halves)
        TERMS = (("h", "h"), ("h", "l"), ("l", "h"))
        TERMS0 = (("h", "h"), ("l", "h"), ("h", "l"))

        qk = {}  # (st, p, m) -> feature-major Q/K tile [128, T]
        vts = {}  # (st, w) -> token-major augmented V (32x scale) [128, H*HA]
        prhs = {}  # (st, w, h) -> P' = exp(S^T) [128, 128]
        atoks = {}  # (st, w) -> token-major attention out (32x) [128, D]
        atts = {}  # (st, "h"/"l") -> feature-major attention fp8 hi/lo (32x)
        ys = {}  # st -> output supertile [128, NCH*T] f32

        def u_qk(st, p, m, terms=None):
            terms = terms or TERMS
            pi = 0 if p == "q" else 1
            ps = ps_proj.tile(
                [128, 512], DT.float32, tag="psproj", name=f"ps{p}{m}_{st}"
            )[:, :T]
            n = 0
            for xs, ws in terms:
                for k2 in range(NK2):
                    nc.tensor.matmul(
                        ps,
                        w3(p, ws)[:, 2 * k2 : 2 * k2 + 2, m * 128 : (m + 1) * 128],
                        x3(st, xs)[:, 2 * k2 : 2 * k2 + 2, :],
                        start=(n == 0),
                        stop=(n == 3 * NK2 - 1),
                        perf_mode=DR,
                    )
                    n += 1
            dst = qkpool.tile([128, T], BF, tag=f"{p}{m}", name=f"{p}{m}_{st}")
            nc.scalar.activation(
                dst[:], ps, AFT.Identity,
                bias=bias_sb[:, pi, m : m + 1], scale=1.0 / WSCALE,
            )
            qk[st, p, m] = dst

        def u_qk_quad0(p, ms):
            # kk-major accumulation across four psum groups (two borrowed
            # from idle ps_s banks) so round 0 consumes streaming chunks
            pi = 0 if p == "q" else 1
            pss = []
            for i, m in enumerate(ms):
                if i < 2:
                    pss.append(
                        ps_proj.tile(
                            [128, 512], DT.float32, tag="psproj", name=f"ps{p}{m}_0"
                        )[:, :T]
                    )
                else:
                    pss.append(
                        ps_s.tile([128, T], DT.float32, tag="pss", name=f"ps{p}{m}_0")
                    )
            for n, (xs, ws) in enumerate(TERMS0):
                for k2 in range(NK2):
                    for i, m in enumerate(ms):
                        nc.tensor.matmul(
                            pss[i],
                            w3(p, ws)[:, 2 * k2 : 2 * k2 + 2, m * 128 : (m + 1) * 128],
                            x3(0, xs)[:, 2 * k2 : 2 * k2 + 2, :],
                            start=(n == 0 and k2 == 0),
                            stop=(n == 2 and k2 == NK2 - 1),
                            perf_mode=DR,
                        )
            for i, m in enumerate(ms):
                dst = qkpool.tile([128, T], BF, tag=f"{p}{m}", name=f"{p}{m}_0")
                nc.scalar.activation(
                    dst[:], pss[i], AFT.Identity,
                    bias=bias_sb[:, pi, m : m + 1], scale=1.0 / WSCALE,
                )
                qk[0, p, m] = dst

        def u_vinit(st, w):
            vt = vtokp.tile([128, H * HA], BF, tag=f"vtok{w}", name=f"vtok{w}_{st}")
            ones_bc = bass.AP(
                tensor=ones_sb.tensor,
                offset=ones_sb.offset,
                ap=[ones_sb.ap[0], [0, H], ones_sb.ap[1]],
            )
            nc.vector.tensor_copy(
                vt[:].rearrange("p (h c) -> p h c", c=HA)[:, :, DK:], ones_bc
            )
            vts[st, w] = vt

        def u_v(st, w, half):
            ps = ps_proj.tile(
                [128, 512], DT.float32, tag="psproj", name=f"psv{w}{half}_{st}"
            )
            n = 0
            for xs, ws in TERMS:
                for k2 in range(NK2):
                    nc.tensor.matmul(
                        ps[:],
                        x3(st, xs)[:, 2 * k2 : 2 * k2 + 2, w * 128 : (w + 1) * 128],
                        w3("v", ws)[:, 2 * k2 : 2 * k2 + 2,
                                    half * 512 : (half + 1) * 512],
                        start=(n == 0),
                        stop=(n == 3 * NK2 - 1),
                        perf_mode=DR,
                    )
                    n += 1
            dst = vts[st, w][:, half * 8 * HA : (half + 1) * 8 * HA]
            nc.vector.tensor_copy(
                dst.rearrange("p (h c) -> p h c", c=HA)[:, :, :DK],
                ps[:].rearrange("p (h c) -> p h c", c=DK),
            )

        def u_s1(st, w, h):
            if h == 0:
                atoks[st, w] = atokp.tile(
                    [128, D], BF, tag=f"atok{w}", name=f"atok{w}_{st}"
                )
            m, off = h // 2, (h % 2) * DK
            ws0 = w * 128
            sp = ps_s.tile([128, 128], DT.float32, tag="pss", name=f"s{w}{h}_{st}")
            nc.tensor.matmul(
                sp[:],
                qk[st, "k", m][off : off + DK, ws0 : ws0 + 128],
                qk[st, "q", m][off : off + DK, ws0 : ws0 + 128],
                start=True,
                stop=True,
            )
            prh = ppool.tile([128, 128], BF, tag="p", name=f"p{w}{h}_{st}")
            nc.scalar.activation(prh[:], sp[:], AFT.Exp, scale=0.125)
            prhs[st, w, h] = prh

        def u_s2(st, w, h):
            pv = ps_pv.tile([128, HA], DT.float32, tag="pspv", name=f"pv{w}{h}_{st}")
            nc.tensor.matmul(
                pv[:],
                prhs[st, w, h][:],
                vts[st, w][:, h * HA : (h + 1) * HA],
                start=True,
                stop=True,
            )
            rinv = rpool.tile([128, 1], DT.float32, tag="rr", name=f"rr{w}{h}_{st}")
            nc.vector.reciprocal(rinv[:], pv[:, DK : DK + 1])
            nc.vector.tensor_scalar_mul(
                atoks[st, w][:, h * DK : (h + 1) * DK], pv[:, :DK], rinv[:]
            )

        def u_tr(st, w, m):
            if w == 0 and m == 0:
                for s in "hl":
                    atts[st, s] = attp.tile(
                        [128, NCH * T], F8, tag=f"att{s}", name=f"att{s}_{st}"
                    )
            tp = ps_tr.tile([128, 128], BF, tag="pstr", name=f"ta{m}{w}_{st}")
            nc.tensor.transpose(
                tp[:], atoks[st, w][:, m * 128 : (m + 1) * 128], id_sb[:]
            )
            # fp8 hi/lo split for the DoubleRow O projection
            hi = atts[st, "h"][:, m * T + w * 128 : m * T + (w + 1) * 128]
            nc.vector.tensor_copy(hi, tp[:])
            nc.vector.scalar_tensor_tensor(
                atts[st, "l"][:, m * T + w * 128 : m * T + (w + 1) * 128],
                tp[:],
                1.0,
                hi,
                op0=mybir.AluOpType.mult,
                op1=mybir.AluOpType.subtract,
            )

        def u_o(st, m):
            if m == 0:
                ys[st] = ypool.tile([128, NCH * T], DT.float32, tag="y", name=f"y_{st}")
            ps = ps_proj.tile(
                [128, 512], DT.float32, tag="psproj", name=f"psy{m}_{st}"
            )[:, :T]
            n = 0
            for xs, ws in TERMS:
                a3 = atts[st, xs][:].rearrange("p (k t) -> p k t", t=T)
                for k2 in range(NK2):
                    nc.tensor.matmul(
                        ps,
                        w3("o", ws)[:, 2 * k2 : 2 * k2 + 2,
                                    m * 128 : (m + 1) * 128],
                        a3[:, 2 * k2 : 2 * k2 + 2, :],
                        start=(n == 0),
                        stop=(n == 3 * NK2 - 1),
                        perf_mode=DR,
                    )
                    n += 1
            # att carries the 32x V scale and wo another 32x
            nc.scalar.activation(
                ys[st][:, m * T : (m + 1) * T], ps, AFT.Identity,
                bias=bias_sb[:, 2, m : m + 1], scale=1.0 / (WSCALE * WSCALE),
            )
            yT3 = yT.rearrange("(k p) t -> p k t", p=128)[:, :, st * T : (st + 1) * T]
            ys3 = ys[st][:].rearrange("p (k t) -> p k t", t=T)
            if st == NST - 1 and m >= NCH - 2:
                nc.sync.dma_start(yT3[:, m : m + 1, :], ys3[:, m : m + 1, :])
            elif m % 2 == 1:
                nc.sync.dma_start(yT3[:, m - 1 : m + 1, :], ys3[:, m - 1 : m + 1, :])

        # ---- emission units, weighted by PE cycles (for the merge only)
        QKW = 3 * NK2 * 128  # 12 DoubleRow instrs x 128 cycles
        VW = 3 * NK2 * 256

        def a_units(st):
            # round st >= 1 carries V(st) plus next supertile's Q/K, so round
            # 0 only needs Q/K of supertiles 0-1 — matched to weight arrival
            out = []
            if st == 0:
                out.append((4 * QKW, lambda: u_qk_quad0("q", (0, 1, 2, 3))))
                out.append((4 * QKW, lambda: u_qk_quad0("q", (4, 5, 6, 7))))
                for m in range(NCH):
                    out.append((QKW, lambda m=m: u_qk(1, "q", m)))
                for m in range(NCH):
                    out.append((QKW, lambda m=m: u_qk(0, "k", m, TERMS0)))
                for m in range(NCH):
                    out.append((QKW, lambda m=m: u_qk(1, "k", m)))
            for w in range(NWIN):
                out.append((0, lambda st=st, w=w: u_vinit(st, w)))
                out.append((VW, lambda st=st, w=w: u_v(st, w, 0)))
                out.append((VW, lambda st=st, w=w: u_v(st, w, 1)))
            if 1 <= st and st + 1 < NST:
                for m in range(NCH):
                    out.append((QKW, lambda st=st, m=m: u_qk(st + 1, "q", m)))
                    out.append((QKW, lambda st=st, m=m: u_qk(st + 1, "k", m)))
            return out

        def b_chain_units(st):
            chains = [(w, h) for w in range(NWIN) for h in range(H)]
            out = []
            ntr = {w: 0 for w in range(NWIN)}  # transposes emitted per window

            def flush_tr(w, upto):
                while ntr[w] < upto:
                    out.append((128, lambda st=st, w=w, m=ntr[w]: u_tr(st, w, m)))
                    ntr[w] += 1

            for i in range(len(chains) + lag):
                if i < len(chains):
                    w, h = chains[i]
                    out.append((128, lambda st=st, w=w, h=h: u_s1(st, w, h)))
                j = i - lag
                if 0 <= j < len(chains):
                    w, h = chains[j]
                    out.append((66, lambda st=st, w=w, h=h: u_s2(st, w, h)))
                    # transpose chunk m needs heads 2m,2m+1 scaled; hold back
                    # tr_lag extra pairs so the DVE writes have landed
                    if h % 2 == 1:
                        flush_tr(w, h // 2 - tr_lag)
                    if (w, h) == (1, 1):
                        flush_tr(0, NCH)
                    if (w, h) == (NWIN - 1, H - 1):
                        flush_tr(NWIN - 1, NCH)
            return out

        def o_units(st):
            return [(QKW, lambda st=st, m=m: u_o(st, m)) for m in range(NCH)]

        def merge_lists(a, b):
            wa = sum(x[0] for x in a) or 1
            wb = sum(x[0] for x in b) or 1
            ca = cb = 0.0
            ia = ib = 0
            out = []
            while ia < len(a) or ib < len(b):
                if ib >= len(b) or (
                    ia < len(a)
                    and (ca + a[ia][0] / 2) * wb <= (cb + b[ib][0] / 2) * wa
                ):
                    ca += a[ia][0]
                    out.append(a[ia])
                    ia += 1
                else:
                    cb += b[ib][0]
                    out.append(b[ib])
                    ib += 1
            return out

        def merge_emit(a, b):
            for _, emit in merge_lists(a, b):
                emit()

        # round st: projections(st) + attention chains(st-1) + O proj(st-1-o_shift)
        for st in range(NST + 1 + o_shift):
            if st >= 1 and st + 1 < NST:
                load_x(st + 1)
            a = a_units(st) if st < NST else []
            ost = st - 1 - o_shift
            if 0 <= ost < NST:
                a = merge_lists(a, o_units(ost))
            b = b_chain_units(st - 1) if 0 <= st - 1 < NST else []
            merge_emit(a, b)

    nc.compile()
    return nc


BUILD_KWARGS = {}


def _get_nc():
    if "nc" not in _NC_CACHE:
        _NC_CACHE["nc"] = _build(**BUILD_KWARGS)
    return _NC_CACHE["nc"]


def _make_in_maps(x, Wq, bq, Wk, bk, Wv, bv, Wo, bo):
    import ml_dtypes

    bf16 = ml_dtypes.bfloat16
    f8 = ml_dtypes.float8_e4m3fn
    x = np.asarray(x, dtype=np.float32)
    xa = np.ascontiguousarray(
        x.reshape(N_CORES, TC, D).transpose(0, 2, 1)
    )  # [8, D, TC] f32
    xh = xa.astype(f8)
    xl = (xa - xh.astype(np.float32)).astype(f8)

    w8 = {}
    for p, Wm in (("q", Wq), ("k", Wk), ("v", Wv)):
        wp = np.ascontiguousarray(np.asarray(Wm, np.float32).T) * WSCALE
        wh = wp.astype(f8)
        wl = (wp - wh.astype(np.float32)).astype(f8)
        w8[p, "h"] = wh
        w8[p, "l"] = wl
    wop = np.ascontiguousarray(np.asarray(Wo, np.float32).T) * WSCALE
    wo_h = wop.astype(f8)
    wo_l = (wop - wo_h.astype(np.float32)).astype(f8)

    # fold V bias into output bias: softmax rows sum to 1
    bo_eff = np.asarray(bo, np.float32) + np.asarray(Wo, np.float32) @ np.asarray(
        bv, np.float32
    )
    bias_pack = np.ascontiguousarray(
        np.stack(
            [np.asarray(bq, np.float32), np.asarray(bk, np.float32), bo_eff], axis=0
        ).reshape(3, NCH, 128).transpose(2, 0, 1)
    )  # [128, 3, NCH]; bias_pack[i, p, m] = b_p[m*128 + i]
    ones = np.ones((128, 2), dtype=bf16)
    ident = np.eye(128, dtype=bf16)
    return [
        {
            "xhT": xh[c],
            "xlT": xl[c],
            **{f"w{p}{s}T": w8[p, s] for p in "qkv" for s in "hl"},
            "wohT": wo_h,
            "wolT": wo_l,
            "bias": bias_pack,
            "ones": ones,
            "ident": ident,
        }
        for c in range(N_CORES)
    ]


def _assemble(results):
    yT = np.stack([results[c]["yT"] for c in range(N_CORES)])  # [8, D, TC]
    return np.ascontiguousarray(yT.transpose(0, 2, 1).reshape(4, 8192, D))


def _run(in_maps, **kwargs):
    return run_bass_kernel_spmd(_get_nc(), in_maps, list(range(N_CORES)), **kwargs)


def kernel(x, Wq, bq, Wk, bk, Wv, bv, Wo, bo):
    in_maps = _make_in_maps(x, Wq, bq, Wk, bk, Wv, bv, Wo, bo)
    res = _run(in_maps)
    return _assemble(res.results)


# revision 44
# speedup vs baseline: 1.0043x; 1.0043x over previous
"""Multi-head local (windowed) attention on 8 Trainium2 NeuronCores.

Reference computation (fp32):
  Q/K/V = x @ W{q,k,v}.T + b{q,k,v}            x: [B=4, L=8192, D=1024]
  per window of 128 tokens, per head (H=16, dk=64):
    S = Q K^T / sqrt(dk); P = softmax(S); att = P V
  out = att @ Wo.T + bo
Sharding: data-parallel over the flattened (B*L) token axis — each of the 8
cores gets 4096 tokens = 32 windows (window boundaries align with the split).
Weights replicated; host pre-transposes x / weights, post-transposes output.

Performance structure (751551ns fp32r baseline -> 436109ns, rel 4.7e-3):
  - All four projections run as fp8e4m3 DoubleRow matmuls (0.5 PE
    cycles/row, two 128-deep contraction tiles per instruction) with a
    3-term error-compensated split: x@W ~= xh@Wh + xh@Wl + xl@Wh where
    xh=fp8(x), xl=fp8(x-xh) and Wh/Wl likewise for 32*W^T (the 32x scale
    keeps the tiny 1/sqrt(D)-scaled weights out of fp8's subnormal range;
    it is divided back out in the bias activations). Measured projection
    error 0.13% — better than bf16's 0.24%. Q/K/V split on the host; the
    attention output's hi/lo split happens on DVE after each transpose
    (copy + scalar_tensor_tensor subtract), and Wo carries its own 32x so
    the y activation de-scales by 1/1024.
  - Attention (scores, exp, PV, transposes) runs in bf16 (1 cycle/row at
    any free size). PSUM accumulation, biases, softmax sums, y stay fp32.
  - Three-stage software pipeline: emission round st interleaves
    projections(st) + attention chains(st-1) + O-projection(st-2) in the
    PE stream (engines execute in program order, so emission order IS the
    schedule). Chain units are merged proportionally by PE weight between
    the dense projection units, with an s1->s2 lag so the S -> exp(ScalarE)
    -> PV round-trip is hidden. Steady-state PE occupancy ~100%.
  - Q/K/O biases ride ScalarE (activation Identity with per-partition bias
    AP + 1/32 de-scale; Exp and Identity share one act table) so DVE only
    does V packing, 1/l normalization and transpose copies.
  - DMAs are few and large, queued in consumption order: x0h | wqh(chunks)
    | bias | x0l | wql | x1 | wk | wv | wo. Round 0 runs Q(0) as two
    kk-major quads (terms ordered by operand arrival, two psum groups
    borrowed from idle ps_s banks) plus Q/K of supertile 1, so the PE
    consumes weight chunks as they stream in; every later round carries
    V(st) + QK(st+1). y leaves in quarters (eighths at the end) so the
    final drain only waits a small store.

Layouts: Q/K feature-major via matmul(W^T, x^T); V token-major via swapped
operands, augmented per head with ones columns [V_h (64) | 1 1] so the PV
matmul also emits softmax sums l[q] at psum columns 64:66; P' = exp(S^T)
serves directly as the PV lhsT; normalization is a per-partition
tensor_scalar multiply by 1/l; only the attention output is PE-transposed
back to feature-major; V bias is folded into the output bias on the host
(softmax rows sum to one). The 32x V scale cancels in the y activation.

Note: independent matmul accumulation groups must NOT share a PSUM bank on
real hardware — psum pools are bank-granular (ps_proj 2 + ps_s 3 + ps_pv 2
+ ps_tr 1 = 8 banks).
"""

import sys

sys.path.insert(0, "/opt/trn_rl_repo")

from contextlib import ExitStack

import numpy as np

import concourse.bass as bass  # noqa: F401
import concourse.tile as tile
from concourse import bacc, mybir
from concourse.bass_utils import run_bass_kernel_spmd

DT = mybir.dt
AFT = mybir.ActivationFunctionType
DR = mybir.MatmulPerfMode.DoubleRow

N_CORES = 8
D = 1024  # model dim
H = 16  # heads
DK = 64  # head dim
W = 128  # window size
TC = 4096  # tokens per core
T = 256  # tokens per supertile (2 windows)
NST = TC // T  # supertiles per core
NWIN = T // W  # windows per supertile
NCH = D // 128  # 128-row feature chunks
NK2 = NCH // 2  # DoubleRow contraction-pair count
HA = DK + 2  # augmented V columns per head: [V_h (64) | ones (2)]
BF = DT.bfloat16
F8 = DT.float8e4
WSCALE = 32.0  # fp8 weight pre-scale (de-scaled in the bias activations)

_NC_CACHE = {}


def _build(lag=3, tr_lag=0, ps_proj_bufs=2, ps_s_bufs=2, ps_pv_bufs=2, ps_tr_bufs=2, o_shift=1, s1w=128):
    """Build + compile the single-core SPMD Bass program."""
    nc = bacc.Bacc("TRN2", target_bir_lowering=False, debug=False, num_devices=N_CORES)

    xT8 = {
        s: nc.dram_tensor(f"x{s}T", [D, TC], F8, kind="ExternalInput").ap()
        for s in "hl"
    }
    wT8 = {
        (p, s): nc.dram_tensor(f"w{p}{s}T", [D, D], F8, kind="ExternalInput").ap()
        for p in "qkv"
        for s in "hl"
    }
    woT8 = {
        s: nc.dram_tensor(f"wo{s}T", [D, D], F8, kind="ExternalInput").ap()
        for s in "hl"
    }
    biasT = nc.dram_tensor("bias", [128, 3, NCH], DT.float32, kind="ExternalInput").ap()
    onesT = nc.dram_tensor("ones", [128, 2], BF, kind="ExternalInput").ap()
    identT = nc.dram_tensor("ident", [128, 128], BF, kind="ExternalInput").ap()
    yT = nc.dram_tensor("yT", [D, TC], DT.float32, kind="ExternalOutput").ap()

    with tile.TileContext(nc) as tc, ExitStack() as ctx:
        wpool = ctx.enter_context(tc.tile_pool(name="w", bufs=1))
        const = ctx.enter_context(tc.tile_pool(name="const", bufs=1))
        xpool = ctx.enter_context(tc.tile_pool(name="x", bufs=2))
        qkpool = ctx.enter_context(tc.tile_pool(name="qk", bufs=3))
        vtokp = ctx.enter_context(tc.tile_pool(name="vtok", bufs=2))
        atokp = ctx.enter_context(tc.tile_pool(name="atok", bufs=2))
        attp = ctx.enter_context(tc.tile_pool(name="attT", bufs=2))
        ppool = ctx.enter_context(tc.tile_pool(name="p", bufs=8))
        rpool = ctx.enter_context(tc.tile_pool(name="r", bufs=8))
        ypool = ctx.enter_context(tc.tile_pool(name="y", bufs=2))
        ps_proj = ctx.enter_context(
            tc.tile_pool(name="ps_proj", bufs=ps_proj_bufs, space="PSUM")
        )
        ps_s = ctx.enter_context(tc.tile_pool(name="ps_s", bufs=ps_s_bufs, space="PSUM"))
        ps_pv = ctx.enter_context(tc.tile_pool(name="ps_pv", bufs=ps_pv_bufs, space="PSUM"))
        ps_tr = ctx.enter_context(tc.tile_pool(name="ps_tr", bufs=ps_tr_bufs, space="PSUM"))

        # ---- resident weights + consts, DMA-ordered by first use
        wt = {}  # ("q"/"k"/"v", "h"/"l") -> fp8 tile; "o" -> bf16 tile

        def alloc_w8(p, s):
            wt[p, s] = wpool.tile([128, NCH * D], F8, tag=f"w{p}{s}", name=f"w{p}{s}")

        def load_w8_chunk(p, s, kk):
            nc.sync.dma_start(
                wt[p, s][:, kk * D : (kk + 1) * D],
                wT8[p, s][kk * 128 : (kk + 1) * 128, :],
            )

        def load_w8(p, s):
            alloc_w8(p, s)
            nc.sync.dma_start(
                wt[p, s][:].rearrange("p (k c) -> p k c", c=D),
                wT8[p, s].rearrange("(k p) c -> p k c", p=128),
            )

        xts = {}  # (st, "h"/"l") -> fp8 tile [128, NCH*T]

        def load_x(st):
            for s in "hl":
                t = xpool.tile([128, NCH * T], F8, tag=f"x{s}", name=f"x{s}_{st}")
                nc.sync.dma_start(
                    t[:].rearrange("p (k t) -> p k t", t=T),
                    xT8[s].rearrange("(k p) t -> p k t", p=128)[
                        :, :, st * T : (st + 1) * T
                    ],
                )
                xts[st, s] = t

        # round 0 consumes wq chunk-wise as it streams in
        alloc_w8("q", "h")
        alloc_w8("q", "l")
        alloc_w8("k", "h")
        alloc_w8("k", "l")
        xts[0, "h"] = xpool.tile([128, NCH * T], F8, tag="xh", name="xh_0")
        xts[0, "l"] = xpool.tile([128, NCH * T], F8, tag="xl", name="xl_0")
        nc.sync.dma_start(
            xts[0, "h"][:].rearrange("p (k t) -> p k t", t=T),
            xT8["h"].rearrange("(k p) t -> p k t", p=128)[:, :, :T],
        )
        for kk in range(3):
            load_w8_chunk("q", "h", kk)
        bias_sb = const.tile([128, 3, NCH], DT.float32, tag="bias")
        nc.sync.dma_start(bias_sb[:], biasT)
        nc.sync.dma_start(
            xts[0, "l"][:].rearrange("p (k t) -> p k t", t=T),
            xT8["l"].rearrange("(k p) t -> p k t", p=128)[:, :, :T],
        )
        for kk in range(3, NCH):
            load_w8_chunk("q", "h", kk)
        for kk in range(NCH):
            load_w8_chunk("q", "l", kk)
        load_x(1)
        nc.sync.dma_start(
            wt["k", "h"][:].rearrange("p (k c) -> p k c", c=D),
            wT8["k", "h"].rearrange("(k p) c -> p k c", p=128),
        )
        nc.sync.dma_start(
            wt["k", "l"][:].rearrange("p (k c) -> p k c", c=D),
            wT8["k", "l"].rearrange("(k p) c -> p k c", p=128),
        )
        ones_sb = const.tile([128, 2], BF, tag="ones")
        nc.sync.dma_start(ones_sb[:], onesT)
        id_sb = const.tile([128, 128], BF, tag="ident")
        nc.sync.dma_start(id_sb[:], identT)
        load_w8("v", "h")
        load_w8("v", "l")
        for s in "hl":
            wt["o", s] = wpool.tile(
                [128, NCH * D], F8, tag=f"wo{s}", name=f"wo{s}"
            )
            nc.sync.dma_start(
                wt["o", s][:].rearrange("p (k c) -> p k c", c=D),
                woT8[s].rearrange("(k p) c -> p k c", p=128),
            )

        def w3(p, s):
            return wt[p, s][:].rearrange("p (k c) -> p k c", c=D)

        def x3(st, s):
            return xts[st, s][:].rearrange("p (k t) -> p k t", t=T)

        # fp8 DoubleRow term list: (x side, w side); TERMS0 matches the
        # round-0 DMA arrival order (x_lo lands before the w_lo halves)
        TERMS = (("h", "h"), ("h", "l"), ("l", "h"))
        TERMS0 = (("h", "h"), ("l", "h"), ("h", "l"))

        qk = {}  # (st, p, m) -> feature-major Q/K tile [128, T]
        vts = {}  # (st, w) -> token-major augmented V (32x scale) [128, H*HA]
        prhs = {}  # (st, w, h) -> P' = exp(S^T) [128, 128]
        atoks = {}  # (st, w) -> token-major attention out (32x) [128, D]
        atts = {}  # (st, "h"/"l") -> feature-major attention fp8 hi/lo (32x)
        ys = {}  # st -> output supertile [128, NCH*T] f32

        def u_qk(st, p, m, terms=None):
            terms = terms or TERMS
            pi = 0 if p == "q" else 1
            ps = ps_proj.tile(
                [128, 512], DT.float32, tag="psproj", name=f"ps{p}{m}_{st}"
            )[:, :T]
            n = 0
            for xs, ws in terms:
                for k2 in range(NK2):
                    nc.tensor.matmul(
                        ps,
                        w3(p, ws)[:, 2 * k2 : 2 * k2 + 2, m * 128 : (m + 1) * 128],
                        x3(st, xs)[:, 2 * k2 : 2 * k2 + 2, :],
                        start=(n == 0),
                        stop=(n == 3 * NK2 - 1),
                        perf_mode=DR,
                    )
                    n += 1
            dst = qkpool.tile([128, T], BF, tag=f"{p}{m}", name=f"{p}{m}_{st}")
            nc.scalar.activation(
                dst[:], ps, AFT.Identity,
                bias=bias_sb[:, pi, m : m + 1], scale=1.0 / WSCALE,
            )
            qk[st, p, m] = dst

        def u_qk_quad0(p, ms):
            # kk-major accumulation across four psum groups (two borrowed
            # from idle ps_s banks) so round 0 consumes streaming chunks
            pi = 0 if p == "q" else 1
            pss = []
            for i, m in enumerate(ms):
                if i < 2:
                    pss.append(
                        ps_proj.tile(
                            [128, 512], DT.float32, tag="psproj", name=f"ps{p}{m}_0"
                        )[:, :T]
                    )
                else:
                    pss.append(
                        ps_s.tile([128, T], DT.float32, tag="pss", name=f"ps{p}{m}_0")
                    )
            for n, (xs, ws) in enumerate(TERMS0):
                for k2 in range(NK2):
                    for i, m in enumerate(ms):
                        nc.tensor.matmul(
                            pss[i],
                            w3(p, ws)[:, 2 * k2 : 2 * k2 + 2, m * 128 : (m + 1) * 128],
                            x3(0, xs)[:, 2 * k2 : 2 * k2 + 2, :],
                            start=(n == 0 and k2 == 0),
                            stop=(n == 2 and k2 == NK2 - 1),
                            perf_mode=DR,
                        )
            for i, m in enumerate(ms):
                dst = qkpool.tile([128, T], BF, tag=f"{p}{m}", name=f"{p}{m}_0")
                nc.scalar.activation(
                    dst[:], pss[i], AFT.Identity,
                    bias=bias_sb[:, pi, m : m + 1], scale=1.0 / WSCALE,
                )
                qk[0, p, m] = dst

        def u_vinit(st, w):
            vt = vtokp.tile([128, H * HA], BF, tag=f"vtok{w}", name=f"vtok{w}_{st}")
            ones_bc = bass.AP(
                tensor=ones_sb.tensor,
                offset=ones_sb.offset,
                ap=[ones_sb.ap[0], [0, H], ones_sb.ap[1]],
            )
            nc.vector.tensor_copy(
                vt[:].rearrange("p (h c) -> p h c", c=HA)[:, :, DK:], ones_bc
            )
            vts[st, w] = vt

        def u_v(st, w, half):
            ps = ps_proj.tile(
                [128, 512], DT.float32, tag="psproj", name=f"psv{w}{half}_{st}"
            )
            n = 0
            for xs, ws in TERMS:
                for k2 in range(NK2):
                    nc.tensor.matmul(
                        ps[:],
                        x3(st, xs)[:, 2 * k2 : 2 * k2 + 2, w * 128 : (w + 1) * 128],
                        w3("v", ws)[:, 2 * k2 : 2 * k2 + 2,
                                    half * 512 : (half + 1) * 512],
                        start=(n == 0),
                        stop=(n == 3 * NK2 - 1),
                        perf_mode=DR,
                    )
                    n += 1
            dst = vts[st, w][:, half * 8 * HA : (half + 1) * 8 * HA]
            nc.vector.tensor_copy(
                dst.rearrange("p (h c) -> p h c", c=HA)[:, :, :DK],
                ps[:].rearrange("p (h c) -> p h c", c=DK),
            )

        def u_s1(st, w, h):
            if h == 0:
                atoks[st, w] = atokp.tile(
                    [128, D], BF, tag=f"atok{w}", name=f"atok{w}_{st}"
                )
            m, off = h // 2, (h % 2) * DK
            ws0 = w * 128
            sp = ps_s.tile([128, 128], DT.float32, tag="pss", name=f"s{w}{h}_{st}")
            nc.tensor.matmul(
                sp[:],
                qk[st, "k", m][off : off + DK, ws0 : ws0 + 128],
                qk[st, "q", m][off : off + DK, ws0 : ws0 + 128],
                start=True,
                stop=True,
            )
            prh = ppool.tile([128, 128], BF, tag="p", name=f"p{w}{h}_{st}")
            nc.scalar.activation(prh[:], sp[:], AFT.Exp, scale=0.125)
            prhs[st, w, h] = prh

        def u_s2(st, w, h):
            pv = ps_pv.tile([128, HA], DT.float32, tag="pspv", name=f"pv{w}{h}_{st}")
            nc.tensor.matmul(
                pv[:],
                prhs[st, w, h][:],
                vts[st, w][:, h * HA : (h + 1) * HA],
                start=True,
                stop=True,
            )
            rinv = rpool.tile([128, 1], DT.float32, tag="rr", name=f"rr{w}{h}_{st}")
            nc.vector.reciprocal(rinv[:], pv[:, DK : DK + 1])
            nc.vector.tensor_scalar_mul(
                atoks[st, w][:, h * DK : (h + 1) * DK], pv[:, :DK], rinv[:]
            )

        def u_tr(st, w, m):
            if w == 0 and m == 0:
                for s in "hl":
                    atts[st, s] = attp.tile(
                        [128, NCH * T], F8, tag=f"att{s}", name=f"att{s}_{st}"
                    )
            tp = ps_tr.tile([128, 128], BF, tag="pstr", name=f"ta{m}{w}_{st}")
            nc.tensor.transpose(
                tp[:], atoks[st, w][:, m * 128 : (m + 1) * 128], id_sb[:]
            )
            # fp8 hi/lo split for the DoubleRow O projection
            hi = atts[st, "h"][:, m * T + w * 128 : m * T + (w + 1) * 128]
            nc.vector.tensor_copy(hi, tp[:])
            nc.vector.scalar_tensor_tensor(
                atts[st, "l"][:, m * T + w * 128 : m * T + (w + 1) * 128],
                tp[:],
                1.0,
                hi,
                op0=mybir.AluOpType.mult,
                op1=mybir.AluOpType.subtract,
            )

        def u_o(st, m):
            if m == 0:
                ys[st] = ypool.tile([128, NCH * T], DT.float32, tag="y", name=f"y_{st}")
            ps = ps_proj.tile(
                [128, 512], DT.float32, tag="psproj", name=f"psy{m}_{st}"
            )[:, :T]
            n = 0
            for xs, ws in TERMS:
                a3 = atts[st, xs][:].rearrange("p (k t) -> p k t", t=T)
                for k2 in range(NK2):
                    nc.tensor.matmul(
                        ps,
                        w3("o", ws)[:, 2 * k2 : 2 * k2 + 2,
                                    m * 128 : (m + 1) * 128],
                        a3[:, 2 * k2 : 2 * k2 + 2, :],
                        start=(n == 0),
                        stop=(n == 3 * NK2 - 1),
                        perf_mode=DR,
                    )
                    n += 1
            # att carries the 32x V scale and wo another 32x
            nc.scalar.activation(
                ys[st][:, m * T : (m + 1) * T], ps, AFT.Identity,
                bias=bias_sb[:, 2, m : m + 1], scale=1.0 / (WSCALE * WSCALE),
            )
            yT3 = yT.rearrange("(k p) t -> p k t", p=128)[:, :, st * T : (st + 1) * T]
            ys3 = ys[st][:].rearrange("p (k t) -> p k t", t=T)
            if st == NST - 1 and m >= NCH - 2:
                nc.sync.dma_start(yT3[:, m : m + 1, :], ys3[:, m : m + 1, :])
            elif m % 2 == 1:
                nc.sync.dma_start(yT3[:, m - 1 : m + 1, :], ys3[:, m - 1 : m + 1, :])

        # ---- emission units, weighted by PE cycles (for the merge only)
        QKW = 3 * NK2 * 128  # 12 DoubleRow instrs x 128 cycles
        VW = 3 * NK2 * 256

        def a_units(st):
            # round st >= 1 carries V(st) plus next supertile's Q/K, so round
            # 0 only needs Q/K of supertiles 0-1 — matched to weight arrival
            out = []
            if st == 0:
                out.append((4 * QKW, lambda: u_qk_quad0("q", (0, 1, 2, 3))))
                out.append((4 * QKW, lambda: u_qk_quad0("q", (4, 5, 6, 7))))
                for m in range(NCH):
                    out.append((QKW, lambda m=m: u_qk(1, "q", m)))
                for m in range(NCH):
                    out.append((QKW, lambda m=m: u_qk(0, "k", m, TERMS0)))
                for m in range(NCH):
                    out.append((QKW, lambda m=m: u_qk(1, "k", m)))
            for w in range(NWIN):
                out.append((0, lambda st=st, w=w: u_vinit(st, w)))
                out.append((VW, lambda st=st, w=w: u_v(st, w, 0)))
                out.append((VW, lambda st=st, w=w: u_v(st, w, 1)))
            if 1 <= st and st + 1 < NST:
                for m in range(NCH):
                    out.append((QKW, lambda st=st, m=m: u_qk(st + 1, "q", m)))
                    out.append((QKW, lambda st=st, m=m: u_qk(st + 1, "k", m)))
            return out

        def b_chain_units(st):
            chains = [(w, h) for w in range(NWIN) for h in range(H)]
            out = []
            ntr = {w: 0 for w in range(NWIN)}  # transposes emitted per window

            def flush_tr(w, upto):
                while ntr[w] < upto:
                    out.append((128, lambda st=st, w=w, m=ntr[w]: u_tr(st, w, m)))
                    ntr[w] += 1

            for i in range(len(chains) + lag):
                if i < len(chains):
                    w, h = chains[i]
                    out.append((s1w, lambda st=st, w=w, h=h: u_s1(st, w, h)))
                j = i - lag
                if 0 <= j < len(chains):
                    w, h = chains[j]
                    out.append((66, lambda st=st, w=w, h=h: u_s2(st, w, h)))
                    # transpose chunk m needs heads 2m,2m+1 scaled; hold back
                    # tr_lag extra pairs so the DVE writes have landed
                    if h % 2 == 1:
                        flush_tr(w, h // 2 - tr_lag)
                    if (w, h) == (1, 1):
                        flush_tr(0, NCH)
                    if (w, h) == (NWIN - 1, H - 1):
                        flush_tr(NWIN - 1, NCH)
            return out

        def o_units(st):
            return [(QKW, lambda st=st, m=m: u_o(st, m)) for m in range(NCH)]

        def merge_lists(a, b):
            wa = sum(x[0] for x in a) or 1
            wb = sum(x[0] for x in b) or 1
            ca = cb = 0.0
            ia = ib = 0
            out = []
            while ia < len(a) or ib < len(b):
                if ib >= len(b) or (
                    ia < len(a)
                    and (ca + a[ia][0] / 2) * wb <= (cb + b[ib][0] / 2) * wa
                ):
                    ca += a[ia][0]
                    out.append(a[ia])
                    ia += 1
                else:
                    cb += b[ib][0]
                    out.append(b[ib])
                    ib += 1
            return out

        def merge_emit(a, b):
            for _, emit in merge_lists(a, b):
                emit()

        # round st: projections(st) + attention chains(st-1) + O proj(st-1-o_shift)
        for st in range(NST + 1 + o_shift):
            if st >= 1 and st + 1 < NST:
                load_x(st + 1)
            a = a_units(st) if st < NST else []
            ost = st - 1 - o_shift
            if 0 <= ost < NST:
                a = merge_lists(a, o_units(ost))
            b = b_chain_units(st - 1) if 0 <= st - 1 < NST else []
            merge_emit(a, b)

    nc.compile()
    return nc


BUILD_KWARGS = {}


def _get_nc():
    if "nc" not in _NC_CACHE:
        _NC_CACHE["nc"] = _build(**BUILD_KWARGS)
    return _NC_CACHE["nc"]


def _make_in_maps(x, Wq, bq, Wk, bk, Wv, bv, Wo, bo):
    import ml_dtypes

    bf16 = ml_dtypes.bfloat16
    f8 = ml_dtypes.float8_e4m3fn
    x = np.asarray(x, dtype=np.float32)
    xa = np.ascontiguousarray(
        x.reshape(N_CORES, TC, D).transpose(0, 2, 1)
    )  # [8, D, TC] f32
    xh = xa.astype(f8)
    xl = (xa - xh.astype(np.float32)).astype(f8)

    w8 = {}
    for p, Wm in (("q", Wq), ("k", Wk), ("v", Wv)):
        wp = np.ascontiguousarray(np.asarray(Wm, np.float32).T) * WSCALE
        wh = wp.astype(f8)
        wl = (wp - wh.astype(np.float32)).astype(f8)
        w8[p, "h"] = wh
        w8[p, "l"] = wl
    wop = np.ascontiguousarray(np.asarray(Wo, np.float32).T) * WSCALE
    wo_h = wop.astype(f8)
    wo_l = (wop - wo_h.astype(np.float32)).astype(f8)

    # fold V bias into output bias: softmax rows sum to 1
    bo_eff = np.asarray(bo, np.float32) + np.asarray(Wo, np.float32) @ np.asarray(
        bv, np.float32
    )
    bias_pack = np.ascontiguousarray(
        np.stack(
            [np.asarray(bq, np.float32), np.asarray(bk, np.float32), bo_eff], axis=0
        ).reshape(3, NCH, 128).transpose(2, 0, 1)
    )  # [128, 3, NCH]; bias_pack[i, p, m] = b_p[m*128 + i]
    ones = np.ones((128, 2), dtype=bf16)
    ident = np.eye(128, dtype=bf16)
    return [
        {
            "xhT": xh[c],
            "xlT": xl[c],
            **{f"w{p}{s}T": w8[p, s] for p in "qkv" for s in "hl"},
            "wohT": wo_h,
            "wolT": wo_l,
            "bias": bias_pack,
            "ones": ones,
            "ident": ident,
        }
        for c in range(N_CORES)
    ]


def _assemble(results):
    yT = np.stack([results[c]["yT"] for c in range(N_CORES)])  # [8, D, TC]
    return np.ascontiguousarray(yT.transpose(0, 2, 1).reshape(4, 8192, D))


def _run(in_maps, **kwargs):
    return run_bass_kernel_spmd(_get_nc(), in_maps, list(range(N_CORES)), **kwargs)


def kernel(x, Wq, bq, Wk, bk, Wv, bv, Wo, bo):
    in_maps = _make_in_maps(x, Wq, bq, Wk, bk, Wv, bv, Wo, bo)
    res = _run(in_maps)
    return _assemble(res.results)
